# revision 162
# baseline (speedup 1.0000x reference)
"""Multi-head self-attention (RoPE, causal) Trainium2 kernel, 8-way sharded.

Sharding: data-parallel over batch (B=2) x tensor-parallel over head groups
(16 heads -> 4 groups of 4). Core c handles batch c//4, heads 4*(c%4)..+4.
Each core computes q/k/v projections for its heads, RoPE, causal-softmax
attention, and a Megatron-style row-parallel partial of the output
projection; the host sums the 4 partials per batch.

v3: schedule rebuilt around keeping the PE matmul stream dense. ACT runs
the softmax exp (plus a few copies only where it has slack); the normalized
attention output is rebuilt in c'-major via one xbar dma_start_transpose per
(pair, query-half); out-projection tiles are spread through the late
attention units as PE fillers; early softmax strips of the ACT-bound late
units are hoisted into PE-rich earlier units (scores+exp+mask run early into
held tiles, only the AV stays in-unit); RoPE combines are out-of-place with
the cos-multiply on GPSIMD so the rotate matmul, sin- and cos-multiplies all
overlap; and the startup DMA stream is ordered so the first projection
k-tile lands ~3.5us in with no PE starvation during the k-loop.
"""
import sys
for _p in ("/opt/trn_rl_repo",):
    if _p not in sys.path:
        sys.path.insert(0, _p)

import numpy as np
import ml_dtypes
from contextlib import ExitStack

import concourse.bacc as bacc
import concourse.mybir as mybir
import concourse.tile as tile
from concourse.bass_utils import run_bass_kernel_spmd

F32 = mybir.dt.float32
BF = mybir.dt.bfloat16
AF = mybir.ActivationFunctionType
BF_NP = ml_dtypes.bfloat16

B, T, C = 2, 2048, 1024
H, Dh = 16, 64
HL = 4                      # heads per core
CK = C // 128               # 8 contraction k-tiles for projections
TTL = T // 128              # 16 T-tiles / kv k-tiles
HT = T // 2                 # 1024, the attention q-half width
N_CORES = 8


def build_nc():
    nc = bacc.Bacc("TRN2", target_bir_lowering=False, debug=False, num_devices=N_CORES)

    xt = nc.declare_dram_parameter("xt", [C, T], BF, isOutput=False)
    wqkv = nc.declare_dram_parameter("wqkv", [C, 4 * 128 + HL * Dh], BF, isOutput=False)
    wo = nc.declare_dram_parameter("wo", [HL * Dh, C], BF, isOutput=False)
    cosT = nc.declare_dram_parameter("cosT", [128, T], BF, isOutput=False)
    sinT = nc.declare_dram_parameter("sinT", [128, T], BF, isOutput=False)
    maskT = nc.declare_dram_parameter("maskT", [128, 128], BF, isOutput=False)
    rotT = nc.declare_dram_parameter("rotT", [128, 128], BF, isOutput=False)
    idT = nc.declare_dram_parameter("idT", [128, 128], BF, isOutput=False)
    out = nc.declare_dram_parameter("out", [T, C], BF, isOutput=True)

    with nc.allow_low_precision("bf16 pipeline"), \
         tile.TileContext(nc) as tc, ExitStack() as octx:
        consts = octx.enter_context(tc.tile_pool(name="consts", bufs=1))
        v_pool = octx.enter_context(tc.tile_pool(name="v", bufs=1))
        qkt_pool = octx.enter_context(tc.tile_pool(name="qkt", bufs=1))
        ao_pool = octx.enter_context(tc.tile_pool(name="ao", bufs=1))
        p_pool = octx.enter_context(tc.tile_pool(name="pb", bufs=13))
        hp_pool = octx.enter_context(tc.tile_pool(name="hpb", bufs=21))
        avsb_pool = octx.enter_context(tc.tile_pool(name="avsbp", bufs=1))
        rec_pool = octx.enter_context(tc.tile_pool(name="recp", bufs=6))
        osb_pool = octx.enter_context(tc.tile_pool(name="outsb", bufs=8))
        wo_pool = octx.enter_context(tc.tile_pool(name="wop", bufs=1))
        sc_ps = octx.enter_context(tc.tile_pool(name="scps", bufs=2, space="PSUM"))
        aux_ps = octx.enter_context(tc.tile_pool(name="auxps", bufs=2, space="PSUM"))
        av_ps = octx.enter_context(tc.tile_pool(name="avps", bufs=1, space="PSUM"))

        mask_t = consts.tile([128, 128], BF, tag="mask")
        rotT_t = consts.tile([128, 128], BF, tag="rotT")
        idT_t = consts.tile([128, 128], BF, tag="idT")

        # vext[t]: [128 kpos, 4 heads, 65] with col 64 = ones (denominator)
        vext_t = v_pool.tile([128, TTL, HL, Dh + 1], BF, tag="vext", name="vext")
        vext = [vext_t[:, t_] for t_ in range(TTL)]
        # qkt[mt][half]: mt 0=Q heads01, 1=K heads01, 2=Q heads23, 3=K heads23
        qkt = [[qkt_pool.tile([128, HT], BF, tag=f"qkt{m}_{hf}", name=f"qkt{m}_{hf}")
                for hf in range(2)] for m in range(4)]
        ao = [ao_pool.tile([128, T], BF, tag=f"ao{i}", name=f"ao{i}") for i in range(2)]
        # avsb[(pair, half)]: [128 q, 8 qblocks, 128 c'pair] normalized attn out
        avsb = {(pr_, hf): avsb_pool.tile([128, 8, 128], BF, tag=f"avsb{pr_}{hf}",
                                          name=f"avsb{pr_}{hf}")
                for pr_ in range(2) for hf in range(2)}
        wo_t = [wo_pool.tile([128, C], BF, tag=f"wo{i}", name=f"wo{i}")
                for i in range(2)]

        state = {"pending": None}
        # strips of later (ACT-bound) units whose scores/exp/mask were emitted
        # early inside PE-rich units; (h, half, m) -> (p_tile, cs)
        hoist_store = {}

        def hst(h, half, m):
            """Closure: emit scores+exp(+mask) for strip m of unit (h, half)
            now, into a held tile; the owning unit later runs just the AV."""
            def go():
                qrmt, krmt = (0, 1) if h < 2 else (2, 3)
                pr = 64 * (h % 2)
                q_lo = HT * half
                cs = max(q_lo, 128 * m)
                W = q_lo + HT - cs
                kr_t = qkt[krmt][m // 8]
                kc = 128 * m - HT * (m // 8)
                sc = sc_ps.tile([128, HT], F32, tag="sc", name="sc")
                j = 0
                while 512 * j < W:
                    n = min(512, W - 512 * j)
                    qc = cs - q_lo + 512 * j
                    nc.tensor.matmul(
                        sc[:, 512 * j:512 * j + n],
                        kr_t[pr:pr + 64, kc:kc + 128],
                        qkt[qrmt][half][pr:pr + 64, qc:qc + n],
                        start=True, stop=True)
                    j += 1
                p = hp_pool.tile([128, HT], BF, tag="hp", name="hp")
                nc.scalar.activation(p[:, 0:W], sc[:, 0:W], AF.Exp, scale=0.125)
                if cs == 128 * m:
                    nc.vector.tensor_mul(p[:, 0:128], p[:, 0:128], mask_t[:])
                hoist_store[(h, half, m)] = (p, cs)
            return go

        def emit_normalize(h_, half_, av_):
            """Per-partition-scalar normalize into avsb; on the odd head of a
            pair, one xbar DMA transpose rebuilds the whole [c'pair, half-row]
            of ao from the assembled avsb blocks."""
            pair = h_ // 2
            sb = avsb[(pair, half_)]
            rec = rec_pool.tile([128, 2, 4], F32, tag="rec", name="rec")
            nc.vector.reciprocal(rec[:], av_[:, :, :, Dh])
            for j in range(8):
                nc.vector.tensor_scalar_mul(
                    sb[:, j, 64 * (h_ % 2):64 * (h_ % 2) + 64],
                    av_[:, j // 4, j % 4, 0:Dh],
                    rec[:, j // 4, j % 4:j % 4 + 1])
            if h_ % 2 == 1:
                base = HT * half_
                nc.sync.dma_start_transpose(
                    ao[pair][:, base:base + HT].rearrange("p (j q) -> p j q", j=8),
                    sb[:])

        def attn_unit(h, half, fillers=(), finish=False, fstart=1, prefill=False):
            """scores^T/exp/mask then q-major AV blocks for head h, query half
            `half`. `fillers` are independent emission closures injected
            one-per-strip (from strip `fstart` on) to keep PE fed while the
            softmax pipeline ramps. With `finish` (the very last unit), each
            q-block is normalized, transposed, and its output-projection tile
            emitted the moment its denominator completes."""
            fillers = list(fillers)
            qrmt, krmt = (0, 1) if h < 2 else (2, 3)
            pr = 64 * (h % 2)
            q_lo = HT * half
            n_strips = 8 if half == 0 else 16
            avbox = [None]

            def emit_av(m, p_, cs_, base=0):
                if avbox[0] is None:
                    # allocated lazily (first call is after the pending
                    # normalize of the previous unit ran, so the WAR sems on
                    # the rotated psum buffer are complete)
                    avbox[0] = av_ps.tile([128, 2, 4, Dh + 1], F32, tag="av",
                                          name="av", padded_shape=[128, 2, 4, 128])
                av = avbox[0]
                j0 = max(0, m - 8 * half)
                for j in range(j0, 8):
                    off = base + q_lo + 128 * j - cs_
                    # start only on the first block of each 2KB psum zero
                    # region: start_tensor_calc marks the WHOLE region
                    # pending-zero, so a start per block would discard the
                    # earlier blocks' strip-0 contributions
                    nc.tensor.matmul(
                        av[:, j // 4, j % 4, :],
                        p_[:, off:off + 128],
                        vext[m][:, h, :],
                        start=(m == 0 and j % 4 == 0), stop=(m == j + 8 * half))

            def finish_block(j):
                sb = avsb[(h // 2, half)]
                av = avbox[0]
                recb = rec_pool.tile([128, 1], F32, tag="recb", name="recb")
                nc.vector.reciprocal(recb[:], av[:, j // 4, j % 4, Dh:Dh + 1])
                nc.vector.tensor_scalar_mul(
                    sb[:, j, 64 * (h % 2):64 * (h % 2) + 64],
                    av[:, j // 4, j % 4, 0:Dh], recb[:])
                tpb_pool = sc_ps if j >= 2 else aux_ps
                tpb = tpb_pool.tile([128, 128], BF, tag="sc" if j >= 2 else "aux",
                                    name="tpb")
                nc.tensor.transpose(tpb[:], sb[:, j, :], idT_t[:])
                dst = ao[h // 2][:, HT * half + 128 * j:HT * half + 128 * (j + 1)]
                if j % 2 == 0:
                    nc.vector.tensor_copy(dst, tpb[:])
                else:
                    nc.scalar.copy(dst, tpb[:])
                outproj_tile(8 * half + j,
                             engines=("v", "s") if j < 4 else ("s", "v"),
                             whole_dma=(j < 6))

            if prefill and fillers:
                # one filler ahead of strip 0: covers the boundary stall where
                # ACT is still draining the previous unit's exp backlog
                fillers.pop(0)()
            pending_av = []
            m = 0
            while m < n_strips:
                if (h, half, m) in hoist_store:
                    # scores/exp already ran inside an earlier unit
                    p_h, cs_h = hoist_store.pop((h, half, m))
                    if m == 0 and state["pending"] is not None:
                        emit_normalize(*state["pending"])
                        state["pending"] = None
                    while len(pending_av) >= (3 if finish else (6 if half == 0 else 10)):
                        mm, pp_, cc_, bb_ = pending_av.pop(0)
                        emit_av(mm, pp_, cc_, bb_)
                        if finish and mm - 1 >= 8 * half:
                            finish_block(mm - 1 - 8 * half)
                    pending_av.append((m, p_h, cs_h, 0))
                    if m >= fstart and fillers and (n_strips <= 8 or m % 2 == 1):
                        fillers.pop(0)()
                    m += 1
                    continue
                # the short tail strips of non-finish half-1 units share one
                # sc tile and a single exp (fewer ACT access-latency charges
                # in the ACT-saturated stretch)
                merged = not finish and ((half == 1 and m in (12, 14)) or (half == 0 and m in (4, 6)))
                cs = max(q_lo, 128 * m)
                W = q_lo + HT - cs
                kr_t = qkt[krmt][m // 8]
                kc = 128 * m - HT * (m // 8)
                sc = sc_ps.tile([128, HT], F32, tag="sc", name="sc")
                j = 0
                while 512 * j < W:
                    n = min(512, W - 512 * j)
                    qc = cs - q_lo + 512 * j
                    nc.tensor.matmul(
                        sc[:, 512 * j:512 * j + n],
                        kr_t[pr:pr + 64, kc:kc + 128],
                        qkt[qrmt][half][pr:pr + 64, qc:qc + n],
                        start=True, stop=True)
                    j += 1
                tot = W
                if merged:
                    cs2 = 128 * (m + 1)
                    W2 = q_lo + HT - cs2
                    kc2 = 128 * (m + 1) - HT * ((m + 1) // 8)
                    nc.tensor.matmul(
                        sc[:, W:W + W2],
                        qkt[krmt][(m + 1) // 8][pr:pr + 64, kc2:kc2 + 128],
                        qkt[qrmt][half][pr:pr + 64, cs2 - q_lo:cs2 - q_lo + W2],
                        start=True, stop=True)
                    tot = W + W2
                p = p_pool.tile([128, HT], BF, tag="p", name="p")
                nc.scalar.activation(p[:, 0:tot], sc[:, 0:tot], AF.Exp, scale=0.125)
                if cs == 128 * m:
                    nc.vector.tensor_mul(p[:, 0:128], p[:, 0:128], mask_t[:])
                if merged:
                    nc.vector.tensor_mul(p[:, W:W + 128], p[:, W:W + 128], mask_t[:])
                if m == 0 and state["pending"] is not None:
                    emit_normalize(*state["pending"])
                    state["pending"] = None
                # run AV several strips behind so exp/mask of the producing
                # strip have fully drained by the time PE reaches the AV
                # matmuls; shallow on the finish unit so blocks complete early
                while len(pending_av) >= (3 if finish else (6 if half == 0 else 10)):
                    mm, pp_, cc_, bb_ = pending_av.pop(0)
                    emit_av(mm, pp_, cc_, bb_)
                    # normalize one AV-pop late: block j's DVE recip/mul run
                    # under the NEXT pop's AV matmuls instead of stalling the
                    # transpose right behind its own AV
                    if finish and mm - 1 >= 8 * half:
                        finish_block(mm - 1 - 8 * half)
                pending_av.append((m, p, cs, 0))
                if merged:
                    pending_av.append((m + 1, p, cs2, W))
                if m >= fstart and fillers and (n_strips <= 8 or m % 2 == 1):
                    fillers.pop(0)()
                m += 2 if merged else 1
            for mm, pp_, cc_, bb_ in pending_av:
                emit_av(mm, pp_, cc_, bb_)
                if finish and mm - 1 >= 8 * half:
                    finish_block(mm - 1 - 8 * half)
            if finish:
                finish_block(7)
            if finish:
                state["pending"] = None
            else:
                state["pending"] = (h, half, avbox[0])
            for f in fillers:
                f()

        def copy_out(dst, src, eng):
            if eng == "q":
                half_ = src.shape[-1] // 2
                nc.vector.tensor_copy(dst[:, 0:half_], src[:, 0:half_])
                nc.scalar.copy(dst[:, half_:], src[:, half_:])
            elif eng == "p":
                nc.gpsimd.tensor_copy(dst, src)
            elif eng == "v":
                nc.vector.tensor_copy(dst, src)
            else:
                nc.scalar.copy(dst, src)

        def outproj_tile(t_, engines=("v", "v"), whole_dma=True, quarter=False):
            osb = osb_pool.tile([128, C], BF, tag="osb", name="osb")
            for n in range(2):
                op = aux_ps.tile([128, 512], F32, tag="aux", name="op")
                nc.tensor.matmul(op[:],
                                 ao[0][:, 128 * t_:128 * (t_ + 1)],
                                 wo_t[0][:, 512 * n:512 * (n + 1)],
                                 start=True, stop=False)
                nc.tensor.matmul(op[:],
                                 ao[1][:, 128 * t_:128 * (t_ + 1)],
                                 wo_t[1][:, 512 * n:512 * (n + 1)],
                                 start=False, stop=True)
                if quarter:
                    # drip the tail out in 256-col pieces so the final DMA
                    # departs as early as possible
                    for qq in range(2):
                        lo = 512 * n + 256 * qq
                        copy_out(osb[:, lo:lo + 256], op[:, 256 * qq:256 * qq + 256],
                                 "s" if qq == 0 else "v")
                        nc.sync.dma_start(out[128 * t_:128 * (t_ + 1), lo:lo + 256],
                                          osb[:, lo:lo + 256])
                    continue
                copy_out(osb[:, 512 * n:512 * (n + 1)], op[:], engines[n])
                if not whole_dma:
                    nc.sync.dma_start(out[128 * t_:128 * (t_ + 1), 512 * n:512 * (n + 1)],
                                      osb[:, 512 * n:512 * (n + 1)])
            if whole_dma:
                nc.sync.dma_start(out[128 * t_:128 * (t_ + 1), :], osb[:])

        def ot(t_):
            return lambda: outproj_tile(t_, engines=("v", "v"))

        with tc.tile_pool(name="xtp", bufs=1) as xt_pool, \
             tc.tile_pool(name="wqkp", bufs=1) as wqk_pool, \
             tc.tile_pool(name="ropetab", bufs=1) as rtab_pool, \
             tc.tile_pool(name="ropetmp", bufs=6) as rtmp_pool, \
             tc.tile_pool(name="preq", bufs=6) as pre_pool:

            # single tiles with k as a free dim, so one DMA can carry several
            # k-tiles (fewer HWDGE descriptor-gen serializations)
            wqkv_all = wqk_pool.tile([128, CK, 512 + HL * Dh], BF, tag="wqkv", name="wqkv_all")
            wqkv_t = [wqkv_all[:, k] for k in range(CK)]
            wqk_t = [w[:, 0:512] for w in wqkv_t]
            wv_t = [w[:, 512:512 + HL * Dh] for w in wqkv_t]
            wqkv_r = wqkv.rearrange("(k p) w -> p k w", p=128)

            xt_all = xt_pool.tile([128, CK, T], BF, tag="xt", name="xt_all")
            xt_t = [xt_all[:, k] for k in range(CK)]
            xt_r = xt.rearrange("(k p) w -> p k w", p=128)

            def xt_dma(hf, nchunk=4):
                kc_ = CK // nchunk
                for c in range(nchunk):
                    nc.sync.dma_start(
                        xt_all[:, kc_ * c:kc_ * (c + 1), HT * hf:HT * (hf + 1)],
                        xt_r[:, kc_ * c:kc_ * (c + 1), HT * hf:HT * (hf + 1)])

            # DMA emission order tracks proj_phase0's k-loop: the (wqk m01,
            # xt half0) pair for k-tile 0 first (smallest possible chunks so
            # the first matmul unblocks ~3us in), then k-tiles in growing
            # chunks, then the V weights (vproj follows phase0), rope tables,
            # and the remaining weight columns.
            cos_t = rtab_pool.tile([128, T], BF, tag="cos")
            sin_t = rtab_pool.tile([128, T], BF, tag="sin")
            nc.sync.dma_start(wqkv_all[:, 0:2, 0:256], wqkv_r[:, 0:2, 0:256])
            nc.sync.dma_start(xt_all[:, 0:1, 0:HT], xt_r[:, 0:1, 0:HT])
            nc.sync.dma_start(xt_all[:, 1:2, 0:HT], xt_r[:, 1:2, 0:HT])
            nc.sync.dma_start(wqkv_all[:, 2:8, 0:256], wqkv_r[:, 2:8, 0:256])
            nc.sync.dma_start(xt_all[:, 2:3, 0:HT], xt_r[:, 2:3, 0:HT])
            nc.sync.dma_start(xt_all[:, 3:4, 0:HT], xt_r[:, 3:4, 0:HT])
            nc.sync.dma_start(xt_all[:, 4:5, 0:HT], xt_r[:, 4:5, 0:HT])
            nc.sync.dma_start(xt_all[:, 5:6, 0:HT], xt_r[:, 5:6, 0:HT])
            nc.sync.dma_start(xt_all[:, 6:7, 0:HT], xt_r[:, 6:7, 0:HT])
            nc.sync.dma_start(xt_all[:, 7:8, 0:HT], xt_r[:, 7:8, 0:HT])
            nc.sync.dma_start(wqkv_all[:, 0:4, 512:768], wqkv_r[:, 0:4, 512:768])
            nc.sync.dma_start(wqkv_all[:, 4:8, 512:768], wqkv_r[:, 4:8, 512:768])
            nc.sync.dma_start(mask_t[:], maskT[:])
            nc.sync.dma_start(rotT_t[:], rotT[:])
            nc.sync.dma_start(cos_t[:, 0:HT], cosT[:, 0:HT])
            nc.sync.dma_start(sin_t[:, 0:HT], sinT[:, 0:HT])
            xt_dma(1)
            nc.sync.dma_start(wqkv_all[:, :, 256:512], wqkv_r[:, :, 256:512])
            nc.sync.dma_start(cos_t[:, HT:T], cosT[:, HT:T])
            nc.sync.dma_start(sin_t[:, HT:T], sinT[:, HT:T])
            nc.sync.dma_start(idT_t[:], idT[:])
            for i in range(2):
                nc.sync.dma_start(wo_t[i][:], wo[128 * i:128 * (i + 1), :])
            # denominator ones column, all t-tiles at once
            nc.vector.memset(vext_t[:, :, :, Dh:Dh + 1], 1.0)

            rope_pending = []
            rope_ctr = [0]
            pre_map = {}

            def emit_rope(m, n):
                """rotate-half via a PE permutation matmul, then the cos/sin
                elementwise combine. Out-of-place: reads the pre-rope copy and
                writes the final qkt slice, so the cos-mul runs concurrently
                with the rotate matmul instead of WAR-serializing behind it."""
                dst = qkt[m][n // 2][:, 512 * (n % 2):512 * (n % 2 + 1)]
                src = pre_map.pop((m, n))
                rps = aux_ps.tile([128, 512], F32, tag="aux", name="rps")
                nc.tensor.matmul(rps[:], rotT_t[:], src[:], start=True, stop=True)
                rot = rtmp_pool.tile([128, 512], BF, tag="rot", name="rot")
                nc.vector.tensor_mul(rot[:], rps[:],
                                     sin_t[:, 512 * n:512 * (n + 1)])
                ctr = rope_ctr[0]
                eng = nc.gpsimd
                rope_ctr[0] += 1
                eng.tensor_mul(dst, src[:], cos_t[:, 512 * n:512 * (n + 1)])
                nc.vector.tensor_add(dst, dst, rot[:])

            def flush_rope():
                while rope_pending:
                    emit_rope(*rope_pending.pop(0))

            pp_box = [None]

            def proj_half(m, n, kr, copy_eng):
                if kr[0] == 0:
                    pp_box[0] = aux_ps.tile([128, 512], F32, tag="aux", name="pp")
                pp = pp_box[0]
                for k in range(kr[0], kr[1]):
                    nc.tensor.matmul(pp[:], wqk_t[k][:, 128 * m:128 * (m + 1)],
                                     xt_t[k][:, 512 * n:512 * (n + 1)],
                                     start=(k == 0), stop=(k == CK - 1))
                if kr[1] < CK:
                    return
                pre = pre_pool.tile([128, 512], BF, tag="pre", name="pre")
                copy_out(pre[:], pp[:], copy_eng)
                pre_map[(m, n)] = pre
                rope_pending.append((m, n))
                while len(rope_pending) > 2:
                    emit_rope(*rope_pending.pop(0))

            def proj_group(m, n, copy_eng="p"):
                proj_half(m, n, (0, CK), copy_eng)

            def proj_phase0():
                """First four projection groups k-outer, so matmul k can start
                the moment xt chunk k lands (the 4-group loop consumes k-tiles
                slower than the startup DMA stream delivers them, so PE never
                starves once the first pair arrives). Copies land on three
                different engines; ropes are interleaved with the V-projection
                tiles by the caller so their latency is covered."""
                combos = [(0, 0), (1, 0), (0, 1), (1, 1)]
                pps = [aux_ps.tile([128, 512], F32, tag="aux", name="pp0"),
                       aux_ps.tile([128, 512], F32, tag="aux", name="pp1"),
                       av_ps.tile([128, 512], F32, tag="av", name="pp2"),
                       sc_ps.tile([128, 512], F32, tag="sc", name="pp3")]
                for k in range(CK):
                    for i, (m, n) in enumerate(combos):
                        nc.tensor.matmul(pps[i][:], wqk_t[k][:, 128 * m:128 * (m + 1)],
                                         xt_t[k][:, 512 * n:512 * (n + 1)],
                                         start=(k == 0), stop=(k == CK - 1))
                for i, ((m, n), eng) in enumerate(zip(combos[:3], ("s", "s", "v"))):
                    pre = pre_pool.tile([128, 512], BF, tag="pre", name="pre")
                    copy_out(pre[:], pps[i][:], eng)
                    pre_map[(m, n)] = pre
                return pps[3]

            def vproj_tile(t_, eng=None):
                flush_rope()
                vp = aux_ps.tile([128, HL * Dh], F32, tag="aux", name="vp")
                for k in range(CK):
                    nc.tensor.matmul(vp[:], xt_t[k][:, 128 * t_:128 * (t_ + 1)], wv_t[k][:],
                                     start=(k == 0), stop=(k == CK - 1))
                if eng is None:
                    eng = "v" if t_ % 2 == 0 else "s"
                copy_out(vext[t_][:, :, 0:Dh],
                         vp[:].rearrange("p (h d) -> p h d", h=HL), eng)

            # heads01 projections + V for the first query half, then attention
            # units with the remaining projection work injected between strips
            # (PE executes in emission order, so attention must be emitted as
            # soon as its dependencies are, with later work woven in as filler)
            def pg(m, n, copy_eng="p"):
                return lambda: proj_group(m, n, copy_eng)

            def vt(t_, eng="p"):
                return lambda: vproj_tile(t_, eng)

            pp3 = proj_phase0()
            vproj_tile(0, "s")
            emit_rope(0, 0)
            vproj_tile(1, "s")
            emit_rope(1, 0)
            pre3 = pre_pool.tile([128, 512], BF, tag="pre", name="pre")
            nc.vector.tensor_copy(pre3[:], pp3[:])
            pre_map[(1, 1)] = pre3
            vproj_tile(2, "s")
            emit_rope(0, 1)
            vproj_tile(3, "s")
            emit_rope(1, 1)
            def flush_normalize():
                if state["pending"] is not None:
                    emit_normalize(*state["pending"])
                    state["pending"] = None

            attn_unit(0, 0, [vt(4, "s"), vt(5, "s"), vt(6, "s"), vt(7, "s"),
                             pg(0, 2, "s"), pg(1, 2, "v"), pg(0, 3, "s")])
            flush_rope()
            attn_unit(1, 0, [pg(1, 3, "v"), pg(2, 0, "s"), pg(2, 1, "v"),
                             pg(3, 0, "s"), pg(3, 1, "v"),
                             hst(0, 1, 0), hst(0, 1, 1), hst(0, 1, 2),
                             hst(0, 1, 3)], prefill=True)
            flush_rope()
            attn_unit(2, 0, [vt(8, "s"), vt(9, "v"), vt(10, "s"), vt(11, "v"),
                             pg(2, 2, "v"), hst(0, 1, 4), hst(1, 1, 0),
                             hst(1, 1, 1)], prefill=True)
            attn_unit(3, 0, [pg(2, 3, "v"), hst(1, 1, 2), flush_rope,
                             hst(1, 1, 3)], prefill=True)
            flush_rope()
            attn_unit(0, 1, [vt(12, "v"), hst(1, 1, 4), vt(13, "v"),
                             hst(1, 1, 5), vt(14, "v"), hst(2, 1, 0),
                             vt(15, "v"), hst(2, 1, 1), ot(0), ot(1)],
                      prefill=True)
            attn_unit(1, 1, [pg(3, 2, "v"), ot(2), hst(2, 1, 2), ot(3),
                             hst(2, 1, 3), hst(3, 1, 0), hst(3, 1, 1), ot(4)],
                      prefill=True)
            attn_unit(2, 1, [pg(3, 3, "v"), flush_rope, ot(5), hst(3, 1, 2),
                             ot(6), hst(3, 1, 3), ot(7), hst(3, 1, 4), hst(3, 1, 5)], prefill=True)
            flush_normalize()
            attn_unit(3, 1, fillers=[], finish=True, fstart=1,
                      prefill=True)

    nc.finalize()
    return nc


_NC = None


def _get_nc():
    global _NC
    if _NC is None:
        _NC = build_nc()
    return _NC


def _host_tables():
    inv_freq = 1.0 / (10000.0 ** (np.arange(0, Dh, 2, dtype=np.float32) / Dh))  # [32]
    t = np.arange(T, dtype=np.float32)
    freqs = t[:, None] * inv_freq[None, :]                  # [T, 32]
    emb = np.concatenate([freqs, freqs], axis=-1)           # [T, 64]
    cos = np.cos(emb).T.astype(np.float32)                  # [64, T]
    sin = np.sin(emb).T.astype(np.float32)                  # [64, T]
    sin_signed = sin.copy()
    sin_signed[0:32, :] *= -1.0                             # rotate_half sign fold
    cosT = np.concatenate([cos, cos], axis=0)               # [128, T] two head-halves
    sinT = np.concatenate([sin_signed, sin_signed], axis=0)
    maskT = np.triu(np.ones((128, 128), np.float32))        # keep where k <= q
    sigma = np.empty(64, np.int64)
    sigma[0:32] = 2 * np.arange(32) + 1
    sigma[32:64] = 2 * np.arange(32)
    R = np.zeros((128, 128), np.float32)
    for hh in range(2):
        for d in range(64):
            R[64 * hh + d, 64 * hh + sigma[d]] = 1.0
    rotT = np.ascontiguousarray(R.T)
    idT = np.eye(128, dtype=np.float32)
    return (cosT.astype(BF_NP), sinT.astype(BF_NP), maskT.astype(BF_NP),
            rotT.astype(BF_NP), idT.astype(BF_NP))


def kernel(x, w_qkv, w_out):
    x = np.asarray(x, dtype=np.float32)
    w_qkv = np.asarray(w_qkv, dtype=np.float32)
    w_out = np.asarray(w_out, dtype=np.float32)
    nc = _get_nc()
    cosT, sinT, maskT, rotT, idT = _host_tables()

    in_maps = []
    for core in range(N_CORES):
        b = core // 4
        g = core % 4
        heads = [4 * g + l for l in range(HL)]
        qcols = [w_qkv[:, 64 * h:64 * (h + 1)] for h in heads]
        kcols = [w_qkv[:, C + 64 * h:C + 64 * (h + 1)] for h in heads]
        vcols = [w_qkv[:, 2 * C + 64 * h:2 * C + 64 * (h + 1)] for h in heads]
        # m-tiles: Q01 | K01 | Q23 | K23
        wqkv_loc = np.concatenate(
            [qcols[0], qcols[1], kcols[0], kcols[1], qcols[2], qcols[3], kcols[2], kcols[3]]
            + vcols, axis=1).astype(BF_NP)                  # [C, 768]
        wo_loc = np.concatenate([w_out[64 * h:64 * (h + 1), :] for h in heads],
                                axis=0).astype(BF_NP)
        in_maps.append({
            "xt": np.ascontiguousarray(x[b].T).astype(BF_NP),  # [C, T]
            "wqkv": wqkv_loc,
            "wo": wo_loc,
            "cosT": cosT, "sinT": sinT, "maskT": maskT, "rotT": rotT, "idT": idT,
        })

    # The first execution of a freshly-loaded program image occasionally
    # glitches at the device/runtime level (crash or corrupted output);
    # subsequent executions are deterministic. Retry on crash or
    # non-finite output.
    out_arr = None
    for attempt in range(3):
        try:
            res = run_bass_kernel_spmd(nc, in_maps, core_ids=list(range(N_CORES)))
        except Exception:
            if attempt == 2:
                raise
            continue
        out_arr = np.zeros((B, T, C), np.float32)
        for core in range(N_CORES):
            out_arr[core // 4] += res.results[core]["out"].astype(np.float32)
        if np.isfinite(out_arr).all() and np.abs(out_arr).max() < 1e3:
            break
    return out_arr


# revision 163
# speedup vs baseline: 1.0107x; 1.0107x over previous
"""Multi-head self-attention (RoPE, causal) Trainium2 kernel, 8-way sharded.

Sharding: data-parallel over batch (B=2) x tensor-parallel over head groups
(16 heads -> 4 groups of 4). Core c handles batch c//4, heads 4*(c%4)..+4.
Each core computes q/k/v projections for its heads, RoPE, causal-softmax
attention, and a Megatron-style row-parallel partial of the output
projection; the host sums the 4 partials per batch.

v3: schedule rebuilt around keeping the PE matmul stream dense. ACT runs
the softmax exp (plus a few copies only where it has slack); the normalized
attention output is rebuilt in c'-major via one xbar dma_start_transpose per
(pair, query-half); out-projection tiles are spread through the late
attention units as PE fillers; early softmax strips of the ACT-bound late
units are hoisted into PE-rich earlier units (scores+exp+mask run early into
held tiles, only the AV stays in-unit); RoPE combines are out-of-place with
the cos-multiply on GPSIMD so the rotate matmul, sin- and cos-multiplies all
overlap; and the startup DMA stream is ordered so the first projection
k-tile lands ~3.5us in with no PE starvation during the k-loop.
"""
import sys
for _p in ("/opt/trn_rl_repo",):
    if _p not in sys.path:
        sys.path.insert(0, _p)

import numpy as np
import ml_dtypes
from contextlib import ExitStack

import concourse.bacc as bacc
import concourse.mybir as mybir
import concourse.tile as tile
from concourse.bass_utils import run_bass_kernel_spmd

F32 = mybir.dt.float32
BF = mybir.dt.bfloat16
AF = mybir.ActivationFunctionType
BF_NP = ml_dtypes.bfloat16

B, T, C = 2, 2048, 1024
H, Dh = 16, 64
HL = 4                      # heads per core
CK = C // 128               # 8 contraction k-tiles for projections
TTL = T // 128              # 16 T-tiles / kv k-tiles
HT = T // 2                 # 1024, the attention q-half width
N_CORES = 8


def build_nc():
    nc = bacc.Bacc("TRN2", target_bir_lowering=False, debug=False, num_devices=N_CORES)

    xt = nc.declare_dram_parameter("xt", [C, T], BF, isOutput=False)
    wqkv = nc.declare_dram_parameter("wqkv", [C, 4 * 128 + HL * Dh], BF, isOutput=False)
    wo = nc.declare_dram_parameter("wo", [HL * Dh, C], BF, isOutput=False)
    cosT = nc.declare_dram_parameter("cosT", [128, T], BF, isOutput=False)
    sinT = nc.declare_dram_parameter("sinT", [128, T], BF, isOutput=False)
    maskT = nc.declare_dram_parameter("maskT", [128, 128], BF, isOutput=False)
    rotT = nc.declare_dram_parameter("rotT", [128, 128], BF, isOutput=False)
    idT = nc.declare_dram_parameter("idT", [128, 128], BF, isOutput=False)
    out = nc.declare_dram_parameter("out", [T, C], BF, isOutput=True)

    with nc.allow_low_precision("bf16 pipeline"), \
         tile.TileContext(nc) as tc, ExitStack() as octx:
        consts = octx.enter_context(tc.tile_pool(name="consts", bufs=1))
        v_pool = octx.enter_context(tc.tile_pool(name="v", bufs=1))
        qkt_pool = octx.enter_context(tc.tile_pool(name="qkt", bufs=1))
        ao_pool = octx.enter_context(tc.tile_pool(name="ao", bufs=1))
        p_pool = octx.enter_context(tc.tile_pool(name="pb", bufs=13))
        hp_pool = octx.enter_context(tc.tile_pool(name="hpb", bufs=21))
        avsb_pool = octx.enter_context(tc.tile_pool(name="avsbp", bufs=1))
        rec_pool = octx.enter_context(tc.tile_pool(name="recp", bufs=6))
        osb_pool = octx.enter_context(tc.tile_pool(name="outsb", bufs=8))
        wo_pool = octx.enter_context(tc.tile_pool(name="wop", bufs=1))
        sc_ps = octx.enter_context(tc.tile_pool(name="scps", bufs=2, space="PSUM"))
        aux_ps = octx.enter_context(tc.tile_pool(name="auxps", bufs=2, space="PSUM"))
        av_ps = octx.enter_context(tc.tile_pool(name="avps", bufs=1, space="PSUM"))

        mask_t = consts.tile([128, 128], BF, tag="mask")
        rotT_t = consts.tile([128, 128], BF, tag="rotT")
        idT_t = consts.tile([128, 128], BF, tag="idT")

        # vext[t]: [128 kpos, 4 heads, 65] with col 64 = ones (denominator)
        vext_t = v_pool.tile([128, TTL, HL, Dh + 1], BF, tag="vext", name="vext")
        vext = [vext_t[:, t_] for t_ in range(TTL)]
        # qkt[mt][half]: mt 0=Q heads01, 1=K heads01, 2=Q heads23, 3=K heads23
        qkt = [[qkt_pool.tile([128, HT], BF, tag=f"qkt{m}_{hf}", name=f"qkt{m}_{hf}")
                for hf in range(2)] for m in range(4)]
        ao = [ao_pool.tile([128, T], BF, tag=f"ao{i}", name=f"ao{i}") for i in range(2)]
        # avsb[(pair, half)]: [128 q, 8 qblocks, 128 c'pair] normalized attn out
        avsb = {(pr_, hf): avsb_pool.tile([128, 8, 128], BF, tag=f"avsb{pr_}{hf}",
                                          name=f"avsb{pr_}{hf}")
                for pr_ in range(2) for hf in range(2)}
        wo_t = [wo_pool.tile([128, C], BF, tag=f"wo{i}", name=f"wo{i}")
                for i in range(2)]

        state = {"pending": None}
        # strips of later (ACT-bound) units whose scores/exp/mask were emitted
        # early inside PE-rich units; (h, half, m) -> (p_tile, cs)
        hoist_store = {}

        def hst(h, half, m):
            """Closure: emit scores+exp(+mask) for strip m of unit (h, half)
            now, into a held tile; the owning unit later runs just the AV."""
            def go():
                qrmt, krmt = (0, 1) if h < 2 else (2, 3)
                pr = 64 * (h % 2)
                q_lo = HT * half
                cs = max(q_lo, 128 * m)
                W = q_lo + HT - cs
                kr_t = qkt[krmt][m // 8]
                kc = 128 * m - HT * (m // 8)
                sc = sc_ps.tile([128, HT], F32, tag="sc", name="sc")
                j = 0
                while 512 * j < W:
                    n = min(512, W - 512 * j)
                    qc = cs - q_lo + 512 * j
                    nc.tensor.matmul(
                        sc[:, 512 * j:512 * j + n],
                        kr_t[pr:pr + 64, kc:kc + 128],
                        qkt[qrmt][half][pr:pr + 64, qc:qc + n],
                        start=True, stop=True)
                    j += 1
                p = hp_pool.tile([128, HT], BF, tag="hp", name="hp")
                nc.scalar.activation(p[:, 0:W], sc[:, 0:W], AF.Exp, scale=0.125)
                if cs == 128 * m:
                    nc.vector.tensor_mul(p[:, 0:128], p[:, 0:128], mask_t[:])
                hoist_store[(h, half, m)] = (p, cs)
            return go

        def emit_normalize(h_, half_, av_):
            """Per-partition-scalar normalize into avsb; on the odd head of a
            pair, one xbar DMA transpose rebuilds the whole [c'pair, half-row]
            of ao from the assembled avsb blocks."""
            pair = h_ // 2
            sb = avsb[(pair, half_)]
            rec = rec_pool.tile([128, 2, 4], F32, tag="rec", name="rec")
            nc.vector.reciprocal(rec[:], av_[:, :, :, Dh])
            for j in range(8):
                nc.vector.tensor_scalar_mul(
                    sb[:, j, 64 * (h_ % 2):64 * (h_ % 2) + 64],
                    av_[:, j // 4, j % 4, 0:Dh],
                    rec[:, j // 4, j % 4:j % 4 + 1])
            if h_ % 2 == 1:
                base = HT * half_
                nc.sync.dma_start_transpose(
                    ao[pair][:, base:base + HT].rearrange("p (j q) -> p j q", j=8),
                    sb[:])

        def attn_unit(h, half, fillers=(), finish=False, fstart=1, prefill=False):
            """scores^T/exp/mask then q-major AV blocks for head h, query half
            `half`. `fillers` are independent emission closures injected
            one-per-strip (from strip `fstart` on) to keep PE fed while the
            softmax pipeline ramps. With `finish` (the very last unit), each
            q-block is normalized, transposed, and its output-projection tile
            emitted the moment its denominator completes."""
            fillers = list(fillers)
            qrmt, krmt = (0, 1) if h < 2 else (2, 3)
            pr = 64 * (h % 2)
            q_lo = HT * half
            n_strips = 8 if half == 0 else 16
            avbox = [None]

            def emit_av(m, p_, cs_, base=0):
                if avbox[0] is None:
                    # allocated lazily (first call is after the pending
                    # normalize of the previous unit ran, so the WAR sems on
                    # the rotated psum buffer are complete)
                    avbox[0] = av_ps.tile([128, 2, 4, Dh + 1], F32, tag="av",
                                          name="av", padded_shape=[128, 2, 4, 128])
                av = avbox[0]
                j0 = max(0, m - 8 * half)
                for j in range(j0, 8):
                    off = base + q_lo + 128 * j - cs_
                    # start only on the first block of each 2KB psum zero
                    # region: start_tensor_calc marks the WHOLE region
                    # pending-zero, so a start per block would discard the
                    # earlier blocks' strip-0 contributions
                    nc.tensor.matmul(
                        av[:, j // 4, j % 4, :],
                        p_[:, off:off + 128],
                        vext[m][:, h, :],
                        start=(m == 0 and j % 4 == 0), stop=(m == j + 8 * half))

            def finish_block(j):
                sb = avsb[(h // 2, half)]
                av = avbox[0]
                recb = rec_pool.tile([128, 1], F32, tag="recb", name="recb")
                nc.vector.reciprocal(recb[:], av[:, j // 4, j % 4, Dh:Dh + 1])
                nc.vector.tensor_scalar_mul(
                    sb[:, j, 64 * (h % 2):64 * (h % 2) + 64],
                    av[:, j // 4, j % 4, 0:Dh], recb[:])
                tpb_pool = sc_ps if j >= 2 else aux_ps
                tpb = tpb_pool.tile([128, 128], BF, tag="sc" if j >= 2 else "aux",
                                    name="tpb")
                nc.tensor.transpose(tpb[:], sb[:, j, :], idT_t[:])
                dst = ao[h // 2][:, HT * half + 128 * j:HT * half + 128 * (j + 1)]
                if j % 2 == 0:
                    nc.vector.tensor_copy(dst, tpb[:])
                else:
                    nc.scalar.copy(dst, tpb[:])
                outproj_tile(8 * half + j,
                             engines=("v", "s") if j < 4 else ("s", "v"),
                             whole_dma=(j < 6))

            if prefill and fillers:
                # one filler ahead of strip 0: covers the boundary stall where
                # ACT is still draining the previous unit's exp backlog
                fillers.pop(0)()
            pending_av = []
            m = 0
            while m < n_strips:
                if (h, half, m) in hoist_store:
                    # scores/exp already ran inside an earlier unit
                    p_h, cs_h = hoist_store.pop((h, half, m))
                    if m == 0 and state["pending"] is not None:
                        emit_normalize(*state["pending"])
                        state["pending"] = None
                    while len(pending_av) >= (3 if finish else (6 if half == 0 else 10)):
                        mm, pp_, cc_, bb_ = pending_av.pop(0)
                        emit_av(mm, pp_, cc_, bb_)
                        if finish and mm - 1 >= 8 * half:
                            finish_block(mm - 1 - 8 * half)
                    pending_av.append((m, p_h, cs_h, 0))
                    if m >= fstart and fillers and (n_strips <= 8 or m % 2 == 1):
                        fillers.pop(0)()
                    m += 1
                    continue
                # the short tail strips of non-finish half-1 units share one
                # sc tile and a single exp (fewer ACT access-latency charges
                # in the ACT-saturated stretch)
                merged = not finish and ((half == 1 and m in (12, 14)) or (half == 0 and m in (4, 6)))
                cs = max(q_lo, 128 * m)
                W = q_lo + HT - cs
                kr_t = qkt[krmt][m // 8]
                kc = 128 * m - HT * (m // 8)
                sc = sc_ps.tile([128, HT], F32, tag="sc", name="sc")
                j = 0
                while 512 * j < W:
                    n = min(512, W - 512 * j)
                    qc = cs - q_lo + 512 * j
                    nc.tensor.matmul(
                        sc[:, 512 * j:512 * j + n],
                        kr_t[pr:pr + 64, kc:kc + 128],
                        qkt[qrmt][half][pr:pr + 64, qc:qc + n],
                        start=True, stop=True)
                    j += 1
                tot = W
                if merged:
                    cs2 = 128 * (m + 1)
                    W2 = q_lo + HT - cs2
                    kc2 = 128 * (m + 1) - HT * ((m + 1) // 8)
                    nc.tensor.matmul(
                        sc[:, W:W + W2],
                        qkt[krmt][(m + 1) // 8][pr:pr + 64, kc2:kc2 + 128],
                        qkt[qrmt][half][pr:pr + 64, cs2 - q_lo:cs2 - q_lo + W2],
                        start=True, stop=True)
                    tot = W + W2
                p = p_pool.tile([128, HT], BF, tag="p", name="p")
                nc.scalar.activation(p[:, 0:tot], sc[:, 0:tot], AF.Exp, scale=0.125)
                if cs == 128 * m:
                    nc.vector.tensor_mul(p[:, 0:128], p[:, 0:128], mask_t[:])
                if merged:
                    nc.vector.tensor_mul(p[:, W:W + 128], p[:, W:W + 128], mask_t[:])
                if m == 0 and state["pending"] is not None:
                    emit_normalize(*state["pending"])
                    state["pending"] = None
                # run AV several strips behind so exp/mask of the producing
                # strip have fully drained by the time PE reaches the AV
                # matmuls; shallow on the finish unit so blocks complete early
                while len(pending_av) >= (3 if finish else (6 if half == 0 else 10)):
                    mm, pp_, cc_, bb_ = pending_av.pop(0)
                    emit_av(mm, pp_, cc_, bb_)
                    # normalize one AV-pop late: block j's DVE recip/mul run
                    # under the NEXT pop's AV matmuls instead of stalling the
                    # transpose right behind its own AV
                    if finish and mm - 1 >= 8 * half:
                        finish_block(mm - 1 - 8 * half)
                pending_av.append((m, p, cs, 0))
                if merged:
                    pending_av.append((m + 1, p, cs2, W))
                if m >= fstart and fillers and (n_strips <= 8 or m % 2 == 1):
                    fillers.pop(0)()
                m += 2 if merged else 1
            for mm, pp_, cc_, bb_ in pending_av:
                emit_av(mm, pp_, cc_, bb_)
                if finish and mm - 1 >= 8 * half:
                    finish_block(mm - 1 - 8 * half)
            if finish:
                finish_block(7)
            if finish:
                state["pending"] = None
            else:
                state["pending"] = (h, half, avbox[0])
            for f in fillers:
                f()

        def copy_out(dst, src, eng):
            if eng == "q":
                half_ = src.shape[-1] // 2
                nc.vector.tensor_copy(dst[:, 0:half_], src[:, 0:half_])
                nc.scalar.copy(dst[:, half_:], src[:, half_:])
            elif eng == "p":
                nc.gpsimd.tensor_copy(dst, src)
            elif eng == "v":
                nc.vector.tensor_copy(dst, src)
            else:
                nc.scalar.copy(dst, src)

        def outproj_tile(t_, engines=("v", "v"), whole_dma=True, quarter=False):
            osb = osb_pool.tile([128, C], BF, tag="osb", name="osb")
            for n in range(2):
                op = aux_ps.tile([128, 512], F32, tag="aux", name="op")
                nc.tensor.matmul(op[:],
                                 ao[0][:, 128 * t_:128 * (t_ + 1)],
                                 wo_t[0][:, 512 * n:512 * (n + 1)],
                                 start=True, stop=False)
                nc.tensor.matmul(op[:],
                                 ao[1][:, 128 * t_:128 * (t_ + 1)],
                                 wo_t[1][:, 512 * n:512 * (n + 1)],
                                 start=False, stop=True)
                if quarter:
                    # drip the tail out in 256-col pieces so the final DMA
                    # departs as early as possible
                    for qq in range(2):
                        lo = 512 * n + 256 * qq
                        copy_out(osb[:, lo:lo + 256], op[:, 256 * qq:256 * qq + 256],
                                 "s" if qq == 0 else "v")
                        nc.sync.dma_start(out[128 * t_:128 * (t_ + 1), lo:lo + 256],
                                          osb[:, lo:lo + 256])
                    continue
                copy_out(osb[:, 512 * n:512 * (n + 1)], op[:], engines[n])
                if not whole_dma:
                    nc.sync.dma_start(out[128 * t_:128 * (t_ + 1), 512 * n:512 * (n + 1)],
                                      osb[:, 512 * n:512 * (n + 1)])
            if whole_dma:
                nc.sync.dma_start(out[128 * t_:128 * (t_ + 1), :], osb[:])

        def ot(t_):
            return lambda: outproj_tile(t_, engines=("v", "v"))

        with tc.tile_pool(name="xtp", bufs=1) as xt_pool, \
             tc.tile_pool(name="wqkp", bufs=1) as wqk_pool, \
             tc.tile_pool(name="ropetab", bufs=1) as rtab_pool, \
             tc.tile_pool(name="ropetmp", bufs=6) as rtmp_pool, \
             tc.tile_pool(name="preq", bufs=6) as pre_pool:

            # single tiles with k as a free dim, so one DMA can carry several
            # k-tiles (fewer HWDGE descriptor-gen serializations)
            wqkv_all = wqk_pool.tile([128, CK, 512 + HL * Dh], BF, tag="wqkv", name="wqkv_all")
            wqkv_t = [wqkv_all[:, k] for k in range(CK)]
            wqk_t = [w[:, 0:512] for w in wqkv_t]
            wv_t = [w[:, 512:512 + HL * Dh] for w in wqkv_t]
            wqkv_r = wqkv.rearrange("(k p) w -> p k w", p=128)

            xt_all = xt_pool.tile([128, CK, T], BF, tag="xt", name="xt_all")
            xt_t = [xt_all[:, k] for k in range(CK)]
            xt_r = xt.rearrange("(k p) w -> p k w", p=128)

            def xt_dma(hf, nchunk=4):
                kc_ = CK // nchunk
                for c in range(nchunk):
                    nc.sync.dma_start(
                        xt_all[:, kc_ * c:kc_ * (c + 1), HT * hf:HT * (hf + 1)],
                        xt_r[:, kc_ * c:kc_ * (c + 1), HT * hf:HT * (hf + 1)])

            # DMA emission order tracks proj_phase0's k-loop: the (wqk m01,
            # xt half0) pair for k-tile 0 first (smallest possible chunks so
            # the first matmul unblocks ~3us in), then k-tiles in growing
            # chunks, then the V weights (vproj follows phase0), rope tables,
            # and the remaining weight columns.
            cos_t = rtab_pool.tile([128, T], BF, tag="cos")
            sin_t = rtab_pool.tile([128, T], BF, tag="sin")
            nc.sync.dma_start(wqkv_all[:, 0:2, 0:256], wqkv_r[:, 0:2, 0:256])
            nc.sync.dma_start(xt_all[:, 0:1, 0:HT], xt_r[:, 0:1, 0:HT])
            nc.sync.dma_start(xt_all[:, 1:2, 0:HT], xt_r[:, 1:2, 0:HT])
            nc.sync.dma_start(wqkv_all[:, 2:8, 0:256], wqkv_r[:, 2:8, 0:256])
            nc.sync.dma_start(xt_all[:, 2:3, 0:HT], xt_r[:, 2:3, 0:HT])
            nc.sync.dma_start(xt_all[:, 3:4, 0:HT], xt_r[:, 3:4, 0:HT])
            nc.sync.dma_start(xt_all[:, 4:5, 0:HT], xt_r[:, 4:5, 0:HT])
            nc.sync.dma_start(xt_all[:, 5:6, 0:HT], xt_r[:, 5:6, 0:HT])
            nc.sync.dma_start(xt_all[:, 6:7, 0:HT], xt_r[:, 6:7, 0:HT])
            nc.sync.dma_start(xt_all[:, 7:8, 0:HT], xt_r[:, 7:8, 0:HT])
            nc.sync.dma_start(wqkv_all[:, 0:4, 512:768], wqkv_r[:, 0:4, 512:768])
            nc.sync.dma_start(wqkv_all[:, 4:8, 512:768], wqkv_r[:, 4:8, 512:768])
            nc.sync.dma_start(mask_t[:], maskT[:])
            nc.sync.dma_start(rotT_t[:], rotT[:])
            nc.sync.dma_start(cos_t[:, 0:HT], cosT[:, 0:HT])
            nc.sync.dma_start(sin_t[:, 0:HT], sinT[:, 0:HT])
            xt_dma(1)
            nc.sync.dma_start(wqkv_all[:, :, 256:512], wqkv_r[:, :, 256:512])
            nc.sync.dma_start(cos_t[:, HT:T], cosT[:, HT:T])
            nc.sync.dma_start(sin_t[:, HT:T], sinT[:, HT:T])
            nc.sync.dma_start(idT_t[:], idT[:])
            for i in range(2):
                nc.sync.dma_start(wo_t[i][:], wo[128 * i:128 * (i + 1), :])
            # denominator ones column, all t-tiles at once
            nc.vector.memset(vext_t[:, :, :, Dh:Dh + 1], 1.0)

            rope_pending = []
            rope_ctr = [0]
            pre_map = {}

            def emit_rope(m, n):
                """rotate-half via a PE permutation matmul, then the cos/sin
                elementwise combine. Out-of-place: reads the pre-rope copy and
                writes the final qkt slice, so the cos-mul runs concurrently
                with the rotate matmul instead of WAR-serializing behind it."""
                dst = qkt[m][n // 2][:, 512 * (n % 2):512 * (n % 2 + 1)]
                src = pre_map.pop((m, n))
                rps = aux_ps.tile([128, 512], F32, tag="aux", name="rps")
                nc.tensor.matmul(rps[:], rotT_t[:], src[:], start=True, stop=True)
                rot = rtmp_pool.tile([128, 512], BF, tag="rot", name="rot")
                nc.vector.tensor_mul(rot[:], rps[:],
                                     sin_t[:, 512 * n:512 * (n + 1)])
                ctr = rope_ctr[0]
                eng = nc.gpsimd
                rope_ctr[0] += 1
                eng.tensor_mul(dst, src[:], cos_t[:, 512 * n:512 * (n + 1)])
                nc.vector.tensor_add(dst, dst, rot[:])

            def flush_rope():
                while rope_pending:
                    emit_rope(*rope_pending.pop(0))

            pp_box = [None]

            def proj_half(m, n, kr, copy_eng):
                if kr[0] == 0:
                    pp_box[0] = aux_ps.tile([128, 512], F32, tag="aux", name="pp")
                pp = pp_box[0]
                for k in range(kr[0], kr[1]):
                    nc.tensor.matmul(pp[:], wqk_t[k][:, 128 * m:128 * (m + 1)],
                                     xt_t[k][:, 512 * n:512 * (n + 1)],
                                     start=(k == 0), stop=(k == CK - 1))
                if kr[1] < CK:
                    return
                pre = pre_pool.tile([128, 512], BF, tag="pre", name="pre")
                copy_out(pre[:], pp[:], copy_eng)
                pre_map[(m, n)] = pre
                rope_pending.append((m, n))
                while len(rope_pending) > 2:
                    emit_rope(*rope_pending.pop(0))

            def proj_group(m, n, copy_eng="p"):
                proj_half(m, n, (0, CK), copy_eng)

            def proj_phase0():
                """First four projection groups k-outer, so matmul k can start
                the moment xt chunk k lands (the 4-group loop consumes k-tiles
                slower than the startup DMA stream delivers them, so PE never
                starves once the first pair arrives). Copies land on three
                different engines; ropes are interleaved with the V-projection
                tiles by the caller so their latency is covered."""
                combos = [(0, 0), (1, 0), (0, 1), (1, 1)]
                pps = [aux_ps.tile([128, 512], F32, tag="aux", name="pp0"),
                       aux_ps.tile([128, 512], F32, tag="aux", name="pp1"),
                       av_ps.tile([128, 512], F32, tag="av", name="pp2"),
                       sc_ps.tile([128, 512], F32, tag="sc", name="pp3")]
                for k in range(CK):
                    for i, (m, n) in enumerate(combos):
                        nc.tensor.matmul(pps[i][:], wqk_t[k][:, 128 * m:128 * (m + 1)],
                                         xt_t[k][:, 512 * n:512 * (n + 1)],
                                         start=(k == 0), stop=(k == CK - 1))
                for i, ((m, n), eng) in enumerate(zip(combos[:3], ("s", "s", "v"))):
                    pre = pre_pool.tile([128, 512], BF, tag="pre", name="pre")
                    copy_out(pre[:], pps[i][:], eng)
                    pre_map[(m, n)] = pre
                return pps[3]

            def vproj_tile(t_, eng=None):
                flush_rope()
                vp = aux_ps.tile([128, HL * Dh], F32, tag="aux", name="vp")
                for k in range(CK):
                    nc.tensor.matmul(vp[:], xt_t[k][:, 128 * t_:128 * (t_ + 1)], wv_t[k][:],
                                     start=(k == 0), stop=(k == CK - 1))
                if eng is None:
                    eng = "v" if t_ % 2 == 0 else "s"
                copy_out(vext[t_][:, :, 0:Dh],
                         vp[:].rearrange("p (h d) -> p h d", h=HL), eng)

            # heads01 projections + V for the first query half, then attention
            # units with the remaining projection work injected between strips
            # (PE executes in emission order, so attention must be emitted as
            # soon as its dependencies are, with later work woven in as filler)
            def pg(m, n, copy_eng="p"):
                return lambda: proj_group(m, n, copy_eng)

            def vt(t_, eng="p"):
                return lambda: vproj_tile(t_, eng)

            pp3 = proj_phase0()
            vproj_tile(0, "s")
            emit_rope(0, 0)
            vproj_tile(1, "s")
            emit_rope(1, 0)
            pre3 = pre_pool.tile([128, 512], BF, tag="pre", name="pre")
            nc.vector.tensor_copy(pre3[:], pp3[:])
            pre_map[(1, 1)] = pre3
            vproj_tile(2, "s")
            emit_rope(0, 1)
            vproj_tile(3, "s")
            emit_rope(1, 1)
            def flush_normalize():
                if state["pending"] is not None:
                    emit_normalize(*state["pending"])
                    state["pending"] = None

            attn_unit(0, 0, [vt(4, "s"), vt(5, "s"), vt(6, "s"), vt(7, "s"),
                             pg(0, 2, "s"), pg(1, 2, "v"), pg(0, 3, "s")])
            flush_rope()
            attn_unit(1, 0, [pg(2, 0, "s"), pg(2, 1, "v"),
                             pg(3, 0, "s"), pg(3, 1, "v"),
                             hst(0, 1, 0), hst(0, 1, 1), hst(0, 1, 2),
                             hst(0, 1, 3)], prefill=True)
            flush_rope()
            attn_unit(2, 0, [vt(8, "s"), vt(9, "v"), vt(10, "s"), vt(11, "v"),
                             pg(2, 2, "v"), hst(0, 1, 4), hst(1, 1, 0),
                             hst(1, 1, 1)], prefill=True)
            attn_unit(3, 0, [pg(2, 3, "v"), hst(1, 1, 2), pg(1, 3, "v"),
                             flush_rope, hst(1, 1, 3)], prefill=True)
            flush_rope()
            attn_unit(0, 1, [vt(12, "v"), hst(1, 1, 4), vt(13, "v"),
                             hst(1, 1, 5), vt(14, "v"), hst(2, 1, 0),
                             vt(15, "v"), hst(2, 1, 1), ot(0), ot(1)],
                      prefill=True)
            attn_unit(1, 1, [pg(3, 2, "v"), ot(2), hst(2, 1, 2), ot(3),
                             hst(2, 1, 3), hst(3, 1, 0), hst(3, 1, 1), ot(4)],
                      prefill=True)
            attn_unit(2, 1, [pg(3, 3, "v"), flush_rope, ot(5), hst(3, 1, 2),
                             ot(6), hst(3, 1, 3), ot(7), hst(3, 1, 4), hst(3, 1, 5)], prefill=True)
            flush_normalize()
            attn_unit(3, 1, fillers=[], finish=True, fstart=1,
                      prefill=True)

    nc.finalize()
    return nc


_NC = None


def _get_nc():
    global _NC
    if _NC is None:
        _NC = build_nc()
    return _NC


def _host_tables():
    inv_freq = 1.0 / (10000.0 ** (np.arange(0, Dh, 2, dtype=np.float32) / Dh))  # [32]
    t = np.arange(T, dtype=np.float32)
    freqs = t[:, None] * inv_freq[None, :]                  # [T, 32]
    emb = np.concatenate([freqs, freqs], axis=-1)           # [T, 64]
    cos = np.cos(emb).T.astype(np.float32)                  # [64, T]
    sin = np.sin(emb).T.astype(np.float32)                  # [64, T]
    sin_signed = sin.copy()
    sin_signed[0:32, :] *= -1.0                             # rotate_half sign fold
    cosT = np.concatenate([cos, cos], axis=0)               # [128, T] two head-halves
    sinT = np.concatenate([sin_signed, sin_signed], axis=0)
    maskT = np.triu(np.ones((128, 128), np.float32))        # keep where k <= q
    sigma = np.empty(64, np.int64)
    sigma[0:32] = 2 * np.arange(32) + 1
    sigma[32:64] = 2 * np.arange(32)
    R = np.zeros((128, 128), np.float32)
    for hh in range(2):
        for d in range(64):
            R[64 * hh + d, 64 * hh + sigma[d]] = 1.0
    rotT = np.ascontiguousarray(R.T)
    idT = np.eye(128, dtype=np.float32)
    return (cosT.astype(BF_NP), sinT.astype(BF_NP), maskT.astype(BF_NP),
            rotT.astype(BF_NP), idT.astype(BF_NP))


def kernel(x, w_qkv, w_out):
    x = np.asarray(x, dtype=np.float32)
    w_qkv = np.asarray(w_qkv, dtype=np.float32)
    w_out = np.asarray(w_out, dtype=np.float32)
    nc = _get_nc()
    cosT, sinT, maskT, rotT, idT = _host_tables()

    in_maps = []
    for core in range(N_CORES):
        b = core // 4
        g = core % 4
        heads = [4 * g + l for l in range(HL)]
        qcols = [w_qkv[:, 64 * h:64 * (h + 1)] for h in heads]
        kcols = [w_qkv[:, C + 64 * h:C + 64 * (h + 1)] for h in heads]
        vcols = [w_qkv[:, 2 * C + 64 * h:2 * C + 64 * (h + 1)] for h in heads]
        # m-tiles: Q01 | K01 | Q23 | K23
        wqkv_loc = np.concatenate(
            [qcols[0], qcols[1], kcols[0], kcols[1], qcols[2], qcols[3], kcols[2], kcols[3]]
            + vcols, axis=1).astype(BF_NP)                  # [C, 768]
        wo_loc = np.concatenate([w_out[64 * h:64 * (h + 1), :] for h in heads],
                                axis=0).astype(BF_NP)
        in_maps.append({
            "xt": np.ascontiguousarray(x[b].T).astype(BF_NP),  # [C, T]
            "wqkv": wqkv_loc,
            "wo": wo_loc,
            "cosT": cosT, "sinT": sinT, "maskT": maskT, "rotT": rotT, "idT": idT,
        })

    # The first execution of a freshly-loaded program image occasionally
    # glitches at the device/runtime level (crash or corrupted output);
    # subsequent executions are deterministic. Retry on crash or
    # non-finite output.
    out_arr = None
    for attempt in range(3):
        try:
            res = run_bass_kernel_spmd(nc, in_maps, core_ids=list(range(N_CORES)))
        except Exception:
            if attempt == 2:
                raise
            continue
        out_arr = np.zeros((B, T, C), np.float32)
        for core in range(N_CORES):
            out_arr[core // 4] += res.results[core]["out"].astype(np.float32)
        if np.isfinite(out_arr).all() and np.abs(out_arr).max() < 1e3:
            break
    return out_arr


# revision 164
# speedup vs baseline: 1.0139x; 1.0031x over previous
"""Multi-head self-attention (RoPE, causal) Trainium2 kernel, 8-way sharded.

Sharding: data-parallel over batch (B=2) x tensor-parallel over head groups
(16 heads -> 4 groups of 4). Core c handles batch c//4, heads 4*(c%4)..+4.
Each core computes q/k/v projections for its heads, RoPE, causal-softmax
attention, and a Megatron-style row-parallel partial of the output
projection; the host sums the 4 partials per batch.

v3: schedule rebuilt around keeping the PE matmul stream dense. ACT runs
the softmax exp (plus a few copies only where it has slack); the normalized
attention output is rebuilt in c'-major via one xbar dma_start_transpose per
(pair, query-half); out-projection tiles are spread through the late
attention units as PE fillers; early softmax strips of the ACT-bound late
units are hoisted into PE-rich earlier units (scores+exp+mask run early into
held tiles, only the AV stays in-unit); RoPE combines are out-of-place with
the cos-multiply on GPSIMD so the rotate matmul, sin- and cos-multiplies all
overlap; and the startup DMA stream is ordered so the first projection
k-tile lands ~3.5us in with no PE starvation during the k-loop.
"""
import sys
for _p in ("/opt/trn_rl_repo",):
    if _p not in sys.path:
        sys.path.insert(0, _p)

import numpy as np
import ml_dtypes
from contextlib import ExitStack

import concourse.bacc as bacc
import concourse.mybir as mybir
import concourse.tile as tile
from concourse.bass_utils import run_bass_kernel_spmd

F32 = mybir.dt.float32
BF = mybir.dt.bfloat16
AF = mybir.ActivationFunctionType
BF_NP = ml_dtypes.bfloat16

B, T, C = 2, 2048, 1024
H, Dh = 16, 64
HL = 4                      # heads per core
CK = C // 128               # 8 contraction k-tiles for projections
TTL = T // 128              # 16 T-tiles / kv k-tiles
HT = T // 2                 # 1024, the attention q-half width
N_CORES = 8


def build_nc():
    nc = bacc.Bacc("TRN2", target_bir_lowering=False, debug=False, num_devices=N_CORES)

    xt = nc.declare_dram_parameter("xt", [C, T], BF, isOutput=False)
    wqkv = nc.declare_dram_parameter("wqkv", [C, 4 * 128 + HL * Dh], BF, isOutput=False)
    wo = nc.declare_dram_parameter("wo", [HL * Dh, C], BF, isOutput=False)
    cosT = nc.declare_dram_parameter("cosT", [128, T], BF, isOutput=False)
    sinT = nc.declare_dram_parameter("sinT", [128, T], BF, isOutput=False)
    maskT = nc.declare_dram_parameter("maskT", [128, 128], BF, isOutput=False)
    rotT = nc.declare_dram_parameter("rotT", [128, 128], BF, isOutput=False)
    idT = nc.declare_dram_parameter("idT", [128, 128], BF, isOutput=False)
    out = nc.declare_dram_parameter("out", [T, C], BF, isOutput=True)

    with nc.allow_low_precision("bf16 pipeline"), \
         tile.TileContext(nc) as tc, ExitStack() as octx:
        consts = octx.enter_context(tc.tile_pool(name="consts", bufs=1))
        v_pool = octx.enter_context(tc.tile_pool(name="v", bufs=1))
        qkt_pool = octx.enter_context(tc.tile_pool(name="qkt", bufs=1))
        ao_pool = octx.enter_context(tc.tile_pool(name="ao", bufs=1))
        p_pool = octx.enter_context(tc.tile_pool(name="pb", bufs=13))
        hp_pool = octx.enter_context(tc.tile_pool(name="hpb", bufs=21))
        avsb_pool = octx.enter_context(tc.tile_pool(name="avsbp", bufs=1))
        rec_pool = octx.enter_context(tc.tile_pool(name="recp", bufs=6))
        osb_pool = octx.enter_context(tc.tile_pool(name="outsb", bufs=8))
        wo_pool = octx.enter_context(tc.tile_pool(name="wop", bufs=1))
        sc_ps = octx.enter_context(tc.tile_pool(name="scps", bufs=2, space="PSUM"))
        aux_ps = octx.enter_context(tc.tile_pool(name="auxps", bufs=2, space="PSUM"))
        av_ps = octx.enter_context(tc.tile_pool(name="avps", bufs=1, space="PSUM"))

        mask_t = consts.tile([128, 128], BF, tag="mask")
        rotT_t = consts.tile([128, 128], BF, tag="rotT")
        idT_t = consts.tile([128, 128], BF, tag="idT")

        # vext[t]: [128 kpos, 4 heads, 65] with col 64 = ones (denominator)
        vext_t = v_pool.tile([128, TTL, HL, Dh + 1], BF, tag="vext", name="vext")
        vext = [vext_t[:, t_] for t_ in range(TTL)]
        # qkt[mt][half]: mt 0=Q heads01, 1=K heads01, 2=Q heads23, 3=K heads23
        qkt = [[qkt_pool.tile([128, HT], BF, tag=f"qkt{m}_{hf}", name=f"qkt{m}_{hf}")
                for hf in range(2)] for m in range(4)]
        ao = [ao_pool.tile([128, T], BF, tag=f"ao{i}", name=f"ao{i}") for i in range(2)]
        # avsb[(pair, half)]: [128 q, 8 qblocks, 128 c'pair] normalized attn out
        avsb = {(pr_, hf): avsb_pool.tile([128, 8, 128], BF, tag=f"avsb{pr_}{hf}",
                                          name=f"avsb{pr_}{hf}")
                for pr_ in range(2) for hf in range(2)}
        wo_t = [wo_pool.tile([128, C], BF, tag=f"wo{i}", name=f"wo{i}")
                for i in range(2)]

        state = {"pending": None}
        # strips of later (ACT-bound) units whose scores/exp/mask were emitted
        # early inside PE-rich units; (h, half, m) -> (p_tile, cs)
        hoist_store = {}

        def hst(h, half, m):
            """Closure: emit scores+exp(+mask) for strip m of unit (h, half)
            now, into a held tile; the owning unit later runs just the AV."""
            def go():
                qrmt, krmt = (0, 1) if h < 2 else (2, 3)
                pr = 64 * (h % 2)
                q_lo = HT * half
                cs = max(q_lo, 128 * m)
                W = q_lo + HT - cs
                kr_t = qkt[krmt][m // 8]
                kc = 128 * m - HT * (m // 8)
                sc = sc_ps.tile([128, HT], F32, tag="sc", name="sc")
                j = 0
                while 512 * j < W:
                    n = min(512, W - 512 * j)
                    qc = cs - q_lo + 512 * j
                    nc.tensor.matmul(
                        sc[:, 512 * j:512 * j + n],
                        kr_t[pr:pr + 64, kc:kc + 128],
                        qkt[qrmt][half][pr:pr + 64, qc:qc + n],
                        start=True, stop=True)
                    j += 1
                p = hp_pool.tile([128, HT], BF, tag="hp", name="hp")
                nc.scalar.activation(p[:, 0:W], sc[:, 0:W], AF.Exp, scale=0.125)
                if cs == 128 * m:
                    nc.vector.tensor_mul(p[:, 0:128], p[:, 0:128], mask_t[:])
                hoist_store[(h, half, m)] = (p, cs)
            return go

        def emit_normalize(h_, half_, av_):
            """Per-partition-scalar normalize into avsb; on the odd head of a
            pair, one xbar DMA transpose rebuilds the whole [c'pair, half-row]
            of ao from the assembled avsb blocks."""
            pair = h_ // 2
            sb = avsb[(pair, half_)]
            rec = rec_pool.tile([128, 2, 4], F32, tag="rec", name="rec")
            nc.vector.reciprocal(rec[:], av_[:, :, :, Dh])
            for j in range(8):
                nc.vector.tensor_scalar_mul(
                    sb[:, j, 64 * (h_ % 2):64 * (h_ % 2) + 64],
                    av_[:, j // 4, j % 4, 0:Dh],
                    rec[:, j // 4, j % 4:j % 4 + 1])
            if h_ % 2 == 1:
                base = HT * half_
                nc.sync.dma_start_transpose(
                    ao[pair][:, base:base + HT].rearrange("p (j q) -> p j q", j=8),
                    sb[:])

        def attn_unit(h, half, fillers=(), finish=False, fstart=1, prefill=False):
            """scores^T/exp/mask then q-major AV blocks for head h, query half
            `half`. `fillers` are independent emission closures injected
            one-per-strip (from strip `fstart` on) to keep PE fed while the
            softmax pipeline ramps. With `finish` (the very last unit), each
            q-block is normalized, transposed, and its output-projection tile
            emitted the moment its denominator completes."""
            fillers = list(fillers)
            qrmt, krmt = (0, 1) if h < 2 else (2, 3)
            pr = 64 * (h % 2)
            q_lo = HT * half
            n_strips = 8 if half == 0 else 16
            avbox = [None]

            def emit_av(m, p_, cs_, base=0):
                if avbox[0] is None:
                    # allocated lazily (first call is after the pending
                    # normalize of the previous unit ran, so the WAR sems on
                    # the rotated psum buffer are complete)
                    avbox[0] = av_ps.tile([128, 2, 4, Dh + 1], F32, tag="av",
                                          name="av", padded_shape=[128, 2, 4, 128])
                av = avbox[0]
                j0 = max(0, m - 8 * half)
                for j in range(j0, 8):
                    off = base + q_lo + 128 * j - cs_
                    # start only on the first block of each 2KB psum zero
                    # region: start_tensor_calc marks the WHOLE region
                    # pending-zero, so a start per block would discard the
                    # earlier blocks' strip-0 contributions
                    nc.tensor.matmul(
                        av[:, j // 4, j % 4, :],
                        p_[:, off:off + 128],
                        vext[m][:, h, :],
                        start=(m == 0 and j % 4 == 0), stop=(m == j + 8 * half))

            def finish_block(j):
                sb = avsb[(h // 2, half)]
                av = avbox[0]
                recb = rec_pool.tile([128, 1], F32, tag="recb", name="recb")
                nc.vector.reciprocal(recb[:], av[:, j // 4, j % 4, Dh:Dh + 1])
                nc.vector.tensor_scalar_mul(
                    sb[:, j, 64 * (h % 2):64 * (h % 2) + 64],
                    av[:, j // 4, j % 4, 0:Dh], recb[:])
                tpb_pool = sc_ps if j >= 2 else aux_ps
                tpb = tpb_pool.tile([128, 128], BF, tag="sc" if j >= 2 else "aux",
                                    name="tpb")
                nc.tensor.transpose(tpb[:], sb[:, j, :], idT_t[:])
                dst = ao[h // 2][:, HT * half + 128 * j:HT * half + 128 * (j + 1)]
                if j % 2 == 0:
                    nc.vector.tensor_copy(dst, tpb[:])
                else:
                    nc.scalar.copy(dst, tpb[:])
                outproj_tile(8 * half + j,
                             engines=("v", "s") if j < 4 else ("s", "v"),
                             whole_dma=(j < 6))

            if prefill and fillers:
                # one filler ahead of strip 0: covers the boundary stall where
                # ACT is still draining the previous unit's exp backlog
                fillers.pop(0)()
            pending_av = []
            m = 0
            while m < n_strips:
                if (h, half, m) in hoist_store:
                    # scores/exp already ran inside an earlier unit
                    p_h, cs_h = hoist_store.pop((h, half, m))
                    if m == 0 and state["pending"] is not None:
                        emit_normalize(*state["pending"])
                        state["pending"] = None
                    while len(pending_av) >= (3 if finish else (6 if half == 0 else 10)):
                        mm, pp_, cc_, bb_ = pending_av.pop(0)
                        emit_av(mm, pp_, cc_, bb_)
                        if finish and mm - 1 >= 8 * half:
                            finish_block(mm - 1 - 8 * half)
                    pending_av.append((m, p_h, cs_h, 0))
                    if m >= fstart and fillers and (n_strips <= 8 or m % 2 == 1):
                        fillers.pop(0)()
                    m += 1
                    continue
                # the short tail strips of non-finish half-1 units share one
                # sc tile and a single exp (fewer ACT access-latency charges
                # in the ACT-saturated stretch)
                merged = not finish and ((half == 1 and m in (12, 14)) or (half == 0 and m in (4, 6)))
                cs = max(q_lo, 128 * m)
                W = q_lo + HT - cs
                kr_t = qkt[krmt][m // 8]
                kc = 128 * m - HT * (m // 8)
                sc = sc_ps.tile([128, HT], F32, tag="sc", name="sc")
                j = 0
                while 512 * j < W:
                    n = min(512, W - 512 * j)
                    qc = cs - q_lo + 512 * j
                    nc.tensor.matmul(
                        sc[:, 512 * j:512 * j + n],
                        kr_t[pr:pr + 64, kc:kc + 128],
                        qkt[qrmt][half][pr:pr + 64, qc:qc + n],
                        start=True, stop=True)
                    j += 1
                tot = W
                if merged:
                    cs2 = 128 * (m + 1)
                    W2 = q_lo + HT - cs2
                    kc2 = 128 * (m + 1) - HT * ((m + 1) // 8)
                    nc.tensor.matmul(
                        sc[:, W:W + W2],
                        qkt[krmt][(m + 1) // 8][pr:pr + 64, kc2:kc2 + 128],
                        qkt[qrmt][half][pr:pr + 64, cs2 - q_lo:cs2 - q_lo + W2],
                        start=True, stop=True)
                    tot = W + W2
                p = p_pool.tile([128, HT], BF, tag="p", name="p")
                nc.scalar.activation(p[:, 0:tot], sc[:, 0:tot], AF.Exp, scale=0.125)
                if cs == 128 * m:
                    nc.vector.tensor_mul(p[:, 0:128], p[:, 0:128], mask_t[:])
                if merged:
                    nc.vector.tensor_mul(p[:, W:W + 128], p[:, W:W + 128], mask_t[:])
                if m == 0 and state["pending"] is not None:
                    emit_normalize(*state["pending"])
                    state["pending"] = None
                # run AV several strips behind so exp/mask of the producing
                # strip have fully drained by the time PE reaches the AV
                # matmuls; shallow on the finish unit so blocks complete early
                while len(pending_av) >= (3 if finish else (6 if half == 0 else 10)):
                    mm, pp_, cc_, bb_ = pending_av.pop(0)
                    emit_av(mm, pp_, cc_, bb_)
                    # normalize one AV-pop late: block j's DVE recip/mul run
                    # under the NEXT pop's AV matmuls instead of stalling the
                    # transpose right behind its own AV
                    if finish and mm - 1 >= 8 * half:
                        finish_block(mm - 1 - 8 * half)
                pending_av.append((m, p, cs, 0))
                if merged:
                    pending_av.append((m + 1, p, cs2, W))
                if m >= fstart and fillers and (n_strips <= 8 or m % 2 == 1):
                    fillers.pop(0)()
                m += 2 if merged else 1
            for mm, pp_, cc_, bb_ in pending_av:
                emit_av(mm, pp_, cc_, bb_)
                if finish and mm - 1 >= 8 * half:
                    finish_block(mm - 1 - 8 * half)
            if finish:
                finish_block(7)
            if finish:
                state["pending"] = None
            else:
                state["pending"] = (h, half, avbox[0])
            for f in fillers:
                f()

        def copy_out(dst, src, eng):
            if eng == "q":
                half_ = src.shape[-1] // 2
                nc.vector.tensor_copy(dst[:, 0:half_], src[:, 0:half_])
                nc.scalar.copy(dst[:, half_:], src[:, half_:])
            elif eng == "p":
                nc.gpsimd.tensor_copy(dst, src)
            elif eng == "v":
                nc.vector.tensor_copy(dst, src)
            else:
                nc.scalar.copy(dst, src)

        def outproj_tile(t_, engines=("v", "v"), whole_dma=True, quarter=False):
            osb = osb_pool.tile([128, C], BF, tag="osb", name="osb")
            for n in range(2):
                op = aux_ps.tile([128, 512], F32, tag="aux", name="op")
                nc.tensor.matmul(op[:],
                                 ao[0][:, 128 * t_:128 * (t_ + 1)],
                                 wo_t[0][:, 512 * n:512 * (n + 1)],
                                 start=True, stop=False)
                nc.tensor.matmul(op[:],
                                 ao[1][:, 128 * t_:128 * (t_ + 1)],
                                 wo_t[1][:, 512 * n:512 * (n + 1)],
                                 start=False, stop=True)
                if quarter:
                    # drip the tail out in 256-col pieces so the final DMA
                    # departs as early as possible
                    for qq in range(2):
                        lo = 512 * n + 256 * qq
                        copy_out(osb[:, lo:lo + 256], op[:, 256 * qq:256 * qq + 256],
                                 "s" if qq == 0 else "v")
                        nc.sync.dma_start(out[128 * t_:128 * (t_ + 1), lo:lo + 256],
                                          osb[:, lo:lo + 256])
                    continue
                copy_out(osb[:, 512 * n:512 * (n + 1)], op[:], engines[n])
                if not whole_dma:
                    nc.sync.dma_start(out[128 * t_:128 * (t_ + 1), 512 * n:512 * (n + 1)],
                                      osb[:, 512 * n:512 * (n + 1)])
            if whole_dma:
                nc.sync.dma_start(out[128 * t_:128 * (t_ + 1), :], osb[:])

        def ot(t_):
            return lambda: outproj_tile(t_, engines=("v", "v"))

        with tc.tile_pool(name="xtp", bufs=1) as xt_pool, \
             tc.tile_pool(name="wqkp", bufs=1) as wqk_pool, \
             tc.tile_pool(name="ropetab", bufs=1) as rtab_pool, \
             tc.tile_pool(name="ropetmp", bufs=6) as rtmp_pool, \
             tc.tile_pool(name="preq", bufs=6) as pre_pool:

            # single tiles with k as a free dim, so one DMA can carry several
            # k-tiles (fewer HWDGE descriptor-gen serializations)
            wqkv_all = wqk_pool.tile([128, CK, 512 + HL * Dh], BF, tag="wqkv", name="wqkv_all")
            wqkv_t = [wqkv_all[:, k] for k in range(CK)]
            wqk_t = [w[:, 0:512] for w in wqkv_t]
            wv_t = [w[:, 512:512 + HL * Dh] for w in wqkv_t]
            wqkv_r = wqkv.rearrange("(k p) w -> p k w", p=128)

            xt_all = xt_pool.tile([128, CK, T], BF, tag="xt", name="xt_all")
            xt_t = [xt_all[:, k] for k in range(CK)]
            xt_r = xt.rearrange("(k p) w -> p k w", p=128)

            def xt_dma(hf, nchunk=4):
                kc_ = CK // nchunk
                for c in range(nchunk):
                    nc.sync.dma_start(
                        xt_all[:, kc_ * c:kc_ * (c + 1), HT * hf:HT * (hf + 1)],
                        xt_r[:, kc_ * c:kc_ * (c + 1), HT * hf:HT * (hf + 1)])

            # DMA emission order tracks proj_phase0's k-loop: the (wqk m01,
            # xt half0) pair for k-tile 0 first (smallest possible chunks so
            # the first matmul unblocks ~3us in), then k-tiles in growing
            # chunks, then the V weights (vproj follows phase0), rope tables,
            # and the remaining weight columns.
            cos_t = rtab_pool.tile([128, T], BF, tag="cos")
            sin_t = rtab_pool.tile([128, T], BF, tag="sin")
            nc.sync.dma_start(wqkv_all[:, 0:2, 0:256], wqkv_r[:, 0:2, 0:256])
            nc.sync.dma_start(xt_all[:, 0:1, 0:HT], xt_r[:, 0:1, 0:HT])
            nc.sync.dma_start(xt_all[:, 1:2, 0:HT], xt_r[:, 1:2, 0:HT])
            nc.sync.dma_start(wqkv_all[:, 2:8, 0:256], wqkv_r[:, 2:8, 0:256])
            nc.sync.dma_start(xt_all[:, 2:3, 0:HT], xt_r[:, 2:3, 0:HT])
            nc.sync.dma_start(xt_all[:, 3:4, 0:HT], xt_r[:, 3:4, 0:HT])
            nc.sync.dma_start(xt_all[:, 4:5, 0:HT], xt_r[:, 4:5, 0:HT])
            nc.sync.dma_start(xt_all[:, 5:6, 0:HT], xt_r[:, 5:6, 0:HT])
            nc.sync.dma_start(xt_all[:, 6:7, 0:HT], xt_r[:, 6:7, 0:HT])
            nc.sync.dma_start(xt_all[:, 7:8, 0:HT], xt_r[:, 7:8, 0:HT])
            nc.sync.dma_start(wqkv_all[:, 0:4, 512:768], wqkv_r[:, 0:4, 512:768])
            nc.sync.dma_start(wqkv_all[:, 4:8, 512:768], wqkv_r[:, 4:8, 512:768])
            nc.sync.dma_start(mask_t[:], maskT[:])
            nc.sync.dma_start(rotT_t[:], rotT[:])
            nc.sync.dma_start(cos_t[:, 0:HT], cosT[:, 0:HT])
            nc.sync.dma_start(sin_t[:, 0:HT], sinT[:, 0:HT])
            xt_dma(1)
            nc.sync.dma_start(wqkv_all[:, :, 256:512], wqkv_r[:, :, 256:512])
            nc.sync.dma_start(cos_t[:, HT:T], cosT[:, HT:T])
            nc.sync.dma_start(sin_t[:, HT:T], sinT[:, HT:T])
            nc.sync.dma_start(idT_t[:], idT[:])
            for i in range(2):
                nc.sync.dma_start(wo_t[i][:], wo[128 * i:128 * (i + 1), :])
            # denominator ones column, all t-tiles at once
            nc.vector.memset(vext_t[:, :, :, Dh:Dh + 1], 1.0)

            rope_pending = []
            rope_ctr = [0]
            pre_map = {}

            def emit_rope(m, n):
                """rotate-half via a PE permutation matmul, then the cos/sin
                elementwise combine. Out-of-place: reads the pre-rope copy and
                writes the final qkt slice, so the cos-mul runs concurrently
                with the rotate matmul instead of WAR-serializing behind it."""
                dst = qkt[m][n // 2][:, 512 * (n % 2):512 * (n % 2 + 1)]
                src = pre_map.pop((m, n))
                rps = aux_ps.tile([128, 512], F32, tag="aux", name="rps")
                nc.tensor.matmul(rps[:], rotT_t[:], src[:], start=True, stop=True)
                rot = rtmp_pool.tile([128, 512], BF, tag="rot", name="rot")
                nc.vector.tensor_mul(rot[:], rps[:],
                                     sin_t[:, 512 * n:512 * (n + 1)])
                ctr = rope_ctr[0]
                eng = nc.gpsimd
                rope_ctr[0] += 1
                eng.tensor_mul(dst, src[:], cos_t[:, 512 * n:512 * (n + 1)])
                nc.vector.tensor_add(dst, dst, rot[:])

            def flush_rope():
                while rope_pending:
                    emit_rope(*rope_pending.pop(0))

            pp_box = [None]

            def proj_half(m, n, kr, copy_eng):
                if kr[0] == 0:
                    pp_box[0] = aux_ps.tile([128, 512], F32, tag="aux", name="pp")
                pp = pp_box[0]
                for k in range(kr[0], kr[1]):
                    nc.tensor.matmul(pp[:], wqk_t[k][:, 128 * m:128 * (m + 1)],
                                     xt_t[k][:, 512 * n:512 * (n + 1)],
                                     start=(k == 0), stop=(k == CK - 1))
                if kr[1] < CK:
                    return
                pre = pre_pool.tile([128, 512], BF, tag="pre", name="pre")
                copy_out(pre[:], pp[:], copy_eng)
                pre_map[(m, n)] = pre
                rope_pending.append((m, n))
                while len(rope_pending) > 2:
                    emit_rope(*rope_pending.pop(0))

            def proj_group(m, n, copy_eng="p"):
                proj_half(m, n, (0, CK), copy_eng)

            def proj_phase0():
                """First four projection groups k-outer, so matmul k can start
                the moment xt chunk k lands (the 4-group loop consumes k-tiles
                slower than the startup DMA stream delivers them, so PE never
                starves once the first pair arrives). Copies land on three
                different engines; ropes are interleaved with the V-projection
                tiles by the caller so their latency is covered."""
                combos = [(0, 0), (1, 0), (0, 1), (1, 1)]
                pps = [aux_ps.tile([128, 512], F32, tag="aux", name="pp0"),
                       aux_ps.tile([128, 512], F32, tag="aux", name="pp1"),
                       av_ps.tile([128, 512], F32, tag="av", name="pp2"),
                       sc_ps.tile([128, 512], F32, tag="sc", name="pp3")]
                for k in range(CK):
                    for i, (m, n) in enumerate(combos):
                        nc.tensor.matmul(pps[i][:], wqk_t[k][:, 128 * m:128 * (m + 1)],
                                         xt_t[k][:, 512 * n:512 * (n + 1)],
                                         start=(k == 0), stop=(k == CK - 1))
                for i, ((m, n), eng) in enumerate(zip(combos[:3], ("s", "s", "v"))):
                    pre = pre_pool.tile([128, 512], BF, tag="pre", name="pre")
                    copy_out(pre[:], pps[i][:], eng)
                    pre_map[(m, n)] = pre
                return pps[3]

            def vproj_tile(t_, eng=None):
                flush_rope()
                vp = aux_ps.tile([128, HL * Dh], F32, tag="aux", name="vp")
                for k in range(CK):
                    nc.tensor.matmul(vp[:], xt_t[k][:, 128 * t_:128 * (t_ + 1)], wv_t[k][:],
                                     start=(k == 0), stop=(k == CK - 1))
                if eng is None:
                    eng = "v" if t_ % 2 == 0 else "s"
                copy_out(vext[t_][:, :, 0:Dh],
                         vp[:].rearrange("p (h d) -> p h d", h=HL), eng)

            # heads01 projections + V for the first query half, then attention
            # units with the remaining projection work injected between strips
            # (PE executes in emission order, so attention must be emitted as
            # soon as its dependencies are, with later work woven in as filler)
            def pg(m, n, copy_eng="p"):
                return lambda: proj_group(m, n, copy_eng)

            def vt(t_, eng="p"):
                return lambda: vproj_tile(t_, eng)

            pp3 = proj_phase0()
            vproj_tile(0, "s")
            emit_rope(0, 0)
            vproj_tile(1, "s")
            emit_rope(1, 0)
            pre3 = pre_pool.tile([128, 512], BF, tag="pre", name="pre")
            nc.vector.tensor_copy(pre3[:], pp3[:])
            pre_map[(1, 1)] = pre3
            vproj_tile(2, "s")
            emit_rope(0, 1)
            vproj_tile(3, "s")
            emit_rope(1, 1)
            def flush_normalize():
                if state["pending"] is not None:
                    emit_normalize(*state["pending"])
                    state["pending"] = None

            attn_unit(0, 0, [vt(4, "s"), vt(5, "s"), vt(6, "s"), vt(7, "s"),
                             pg(0, 2, "s"), pg(0, 3, "s")])
            flush_rope()
            attn_unit(1, 0, [pg(2, 0, "s"), pg(2, 1, "v"),
                             pg(3, 0, "s"), pg(3, 1, "v"),
                             hst(0, 1, 0), hst(0, 1, 1), hst(0, 1, 2),
                             hst(0, 1, 3)], prefill=True)
            flush_rope()
            attn_unit(2, 0, [vt(8, "s"), vt(9, "v"), vt(10, "s"), vt(11, "v"),
                             pg(2, 2, "v"), pg(1, 2, "v"), hst(0, 1, 4),
                             hst(1, 1, 0), hst(1, 1, 1)], prefill=True)
            attn_unit(3, 0, [pg(2, 3, "v"), hst(1, 1, 2), pg(1, 3, "v"),
                             flush_rope, hst(1, 1, 3)], prefill=True)
            flush_rope()
            attn_unit(0, 1, [vt(12, "v"), hst(1, 1, 4), vt(13, "v"),
                             hst(1, 1, 5), vt(14, "v"), hst(2, 1, 0),
                             vt(15, "v"), hst(2, 1, 1), ot(0), ot(1)],
                      prefill=True)
            attn_unit(1, 1, [pg(3, 2, "v"), ot(2), hst(2, 1, 2), ot(3),
                             hst(2, 1, 3), hst(3, 1, 0), hst(3, 1, 1), ot(4)],
                      prefill=True)
            attn_unit(2, 1, [pg(3, 3, "v"), flush_rope, ot(5), hst(3, 1, 2),
                             ot(6), hst(3, 1, 3), ot(7), hst(3, 1, 4), hst(3, 1, 5)], prefill=True)
            flush_normalize()
            attn_unit(3, 1, fillers=[], finish=True, fstart=1,
                      prefill=True)

    nc.finalize()
    return nc


_NC = None


def _get_nc():
    global _NC
    if _NC is None:
        _NC = build_nc()
    return _NC


def _host_tables():
    inv_freq = 1.0 / (10000.0 ** (np.arange(0, Dh, 2, dtype=np.float32) / Dh))  # [32]
    t = np.arange(T, dtype=np.float32)
    freqs = t[:, None] * inv_freq[None, :]                  # [T, 32]
    emb = np.concatenate([freqs, freqs], axis=-1)           # [T, 64]
    cos = np.cos(emb).T.astype(np.float32)                  # [64, T]
    sin = np.sin(emb).T.astype(np.float32)                  # [64, T]
    sin_signed = sin.copy()
    sin_signed[0:32, :] *= -1.0                             # rotate_half sign fold
    cosT = np.concatenate([cos, cos], axis=0)               # [128, T] two head-halves
    sinT = np.concatenate([sin_signed, sin_signed], axis=0)
    maskT = np.triu(np.ones((128, 128), np.float32))        # keep where k <= q
    sigma = np.empty(64, np.int64)
    sigma[0:32] = 2 * np.arange(32) + 1
    sigma[32:64] = 2 * np.arange(32)
    R = np.zeros((128, 128), np.float32)
    for hh in range(2):
        for d in range(64):
            R[64 * hh + d, 64 * hh + sigma[d]] = 1.0
    rotT = np.ascontiguousarray(R.T)
    idT = np.eye(128, dtype=np.float32)
    return (cosT.astype(BF_NP), sinT.astype(BF_NP), maskT.astype(BF_NP),
            rotT.astype(BF_NP), idT.astype(BF_NP))


def kernel(x, w_qkv, w_out):
    x = np.asarray(x, dtype=np.float32)
    w_qkv = np.asarray(w_qkv, dtype=np.float32)
    w_out = np.asarray(w_out, dtype=np.float32)
    nc = _get_nc()
    cosT, sinT, maskT, rotT, idT = _host_tables()

    in_maps = []
    for core in range(N_CORES):
        b = core // 4
        g = core % 4
        heads = [4 * g + l for l in range(HL)]
        qcols = [w_qkv[:, 64 * h:64 * (h + 1)] for h in heads]
        kcols = [w_qkv[:, C + 64 * h:C + 64 * (h + 1)] for h in heads]
        vcols = [w_qkv[:, 2 * C + 64 * h:2 * C + 64 * (h + 1)] for h in heads]
        # m-tiles: Q01 | K01 | Q23 | K23
        wqkv_loc = np.concatenate(
            [qcols[0], qcols[1], kcols[0], kcols[1], qcols[2], qcols[3], kcols[2], kcols[3]]
            + vcols, axis=1).astype(BF_NP)                  # [C, 768]
        wo_loc = np.concatenate([w_out[64 * h:64 * (h + 1), :] for h in heads],
                                axis=0).astype(BF_NP)
        in_maps.append({
            "xt": np.ascontiguousarray(x[b].T).astype(BF_NP),  # [C, T]
            "wqkv": wqkv_loc,
            "wo": wo_loc,
            "cosT": cosT, "sinT": sinT, "maskT": maskT, "rotT": rotT, "idT": idT,
        })

    # The first execution of a freshly-loaded program image occasionally
    # glitches at the device/runtime level (crash or corrupted output);
    # subsequent executions are deterministic. Retry on crash or
    # non-finite output.
    out_arr = None
    for attempt in range(3):
        try:
            res = run_bass_kernel_spmd(nc, in_maps, core_ids=list(range(N_CORES)))
        except Exception:
            if attempt == 2:
                raise
            continue
        out_arr = np.zeros((B, T, C), np.float32)
        for core in range(N_CORES):
            out_arr[core // 4] += res.results[core]["out"].astype(np.float32)
        if np.isfinite(out_arr).all() and np.abs(out_arr).max() < 1e3:
            break
    return out_arr
